# revision 10
# baseline (speedup 1.0000x reference)
"""Trainium2 Bass kernel for the CODEXDose-style MoE routing model.

Sharding: pure data-parallel over the batch across 8 NeuronCores (2048 rows
each), all weights replicated. No collectives.

On-device layout: activations are feature-major ("transposed": features on
SBUF partitions, batch on the free axis) so every layer's weights act as the
stationary matmul operand in their natural [K, N] DRAM layout; only the final
512->10000 projection flips to batch-major output (activations stationary) so
the 82 MB result stores with contiguous rows. All matmuls are bf16 with fp32
PSUM accumulation.

Expert routing: argmax/dose are recovered as rowmax of the treatment matrix.
Every expert is computed for every row, but each expert's pre-relu PSUM gets
an additive rank-1 mask (0 for rows routed to it, -1e6 otherwise) injected by
a K=1 matmul; a running elementwise max on VectorE then selects exactly the
routed expert's values, and one final ReLU reproduces relu(x @ W_t + b_t).
"""

from contextlib import ExitStack

import numpy as np
import ml_dtypes

import concourse.bass as bass
import concourse.mybir as mybir
import concourse.tile as tile
from concourse import bacc
from concourse.bass_utils import run_bass_kernel_spmd
from concourse.masks import make_identity

F32 = mybir.dt.float32
BF16 = mybir.dt.bfloat16
AF = mybir.ActivationFunctionType
ALU = mybir.AluOpType

P = 128
NCORES = 8
BATCH = 16384
IN = 5000
INP = 5120          # input features padded to 40 k-tiles of 128
N0 = 512
N1 = 512
N2 = 256
NT = 16
ROWS = BATCH // NCORES      # 2048 rows per core
CHUNK = 512                 # batch columns per matmul (rhs free dim)
K1 = INP // P               # 40 input k-tiles
OUTW = 2 * IN               # 10000
NW = 500                    # dec3 output tile width (20 per row block)
NEG = -1.0e6
QK = 8                      # input k-tiles per load slice
QW = QK * P                 # 1280 input columns per quarter

LAST_EXEC_NS = None
LAST_RESULTS = None


def build_program(has_eb, has_b3, nchunk):
    rows = nchunk * CHUNK
    nc = bacc.Bacc("TRN2", target_bir_lowering=False, debug=False,
                   num_devices=NCORES, num_swdge_queues=4)

    x_d = nc.dram_tensor("x", [rows, INP], F32, kind="ExternalInput").ap()
    td_d = nc.dram_tensor("td", [rows, NT], F32, kind="ExternalInput").ap()
    w1_d = nc.dram_tensor("w1", [INP, N0], BF16, kind="ExternalInput").ap()
    b1_d = nc.dram_tensor("b1", [N0], F32, kind="ExternalInput").ap()
    w2_d = nc.dram_tensor("w2", [N0, N2], BF16, kind="ExternalInput").ap()
    b2_d = nc.dram_tensor("b2", [N2], F32, kind="ExternalInput").ap()
    ew_d = nc.dram_tensor("ew", [NT, N2 + 1, N2], BF16, kind="ExternalInput").ap()
    wd1_d = nc.dram_tensor("wd1", [N2, N1], BF16, kind="ExternalInput").ap()
    c1_d = nc.dram_tensor("c1", [N1], F32, kind="ExternalInput").ap()
    wd2_d = nc.dram_tensor("wd2", [N1, N0], BF16, kind="ExternalInput").ap()
    c2_d = nc.dram_tensor("c2", [N0], F32, kind="ExternalInput").ap()
    wd3_d = nc.dram_tensor("wd3", [N0, OUTW], BF16, kind="ExternalInput").ap()
    selm_d = nc.dram_tensor("selm", [1 + NT, NT * P], BF16,
                            kind="ExternalInput").ap()
    ebr_d = (nc.dram_tensor("ebr", [1, NT * N2], BF16, kind="ExternalInput").ap()
             if has_eb else None)
    b3r_d = (nc.dram_tensor("b3r", [1, OUTW], BF16, kind="ExternalInput").ap()
             if has_b3 else None)
    rec_d = nc.dram_tensor("rec", [rows, OUTW], F32, kind="ExternalOutput").ap()
    lat_d = nc.dram_tensor("lat", [rows, N2], F32, kind="ExternalOutput").ap()

    with tile.TileContext(nc) as tc, ExitStack() as ctx:
        pool = lambda name, bufs, space="SBUF": ctx.enter_context(
            tc.tile_pool(name=name, bufs=bufs, space=space))
        cp = pool("const", 1)

        # ---- resident weights / constants (loaded once) ----
        w1r = cp.tile([P, K1, N0], BF16, tag="w1r")
        for k in range(0, K1, 10):
            nc.sync.dma_start(
                w1r[:, k:k + 10, :],
                w1_d[k * P:(k + 10) * P, :].rearrange("(k p) n -> p k n", p=P))
        w2r = cp.tile([P, 4, N2], BF16, tag="w2r")
        nc.sync.dma_start(w2r[:], w2_d.rearrange("(k p) n -> p k n", p=P))
        ewr = cp.tile([P, NT, 2, N2], BF16, tag="ewr")
        for k in range(2):
            nc.sync.dma_start(
                ewr[:, :, k, :],
                ew_d[:, k * P:(k + 1) * P, :].rearrange("t p n -> p t n"))
        wd1r = cp.tile([P, 2, N1], BF16, tag="wd1r")
        nc.sync.dma_start(wd1r[:], wd1_d.rearrange("(k p) n -> p k n", p=P))
        wd2r = cp.tile([P, 4, N0], BF16, tag="wd2r")
        nc.sync.dma_start(wd2r[:], wd2_d.rearrange("(k p) n -> p k n", p=P))
        wd3r = cp.tile([P, 4, OUTW], BF16, tag="wd3r")
        for k in range(4):
            nc.sync.dma_start(wd3r[:, k, 0:OUTW // 2],
                              wd3_d[k * P:(k + 1) * P, 0:OUTW // 2])
            nc.sync.dma_start(wd3r[:, k, OUTW // 2:OUTW],
                              wd3_d[k * P:(k + 1) * P, OUTW // 2:OUTW])
        b1r = cp.tile([P, 4], F32, tag="b1r")
        nc.sync.dma_start(b1r[:], b1_d.rearrange("(m p) -> p m", p=P))
        b2r = cp.tile([P, 2], F32, tag="b2r")
        nc.sync.dma_start(b2r[:], b2_d.rearrange("(m p) -> p m", p=P))
        c1r = cp.tile([P, 4], F32, tag="c1r")
        nc.sync.dma_start(c1r[:], c1_d.rearrange("(m p) -> p m", p=P))
        c2r = cp.tile([P, 4], F32, tag="c2r")
        nc.sync.dma_start(c2r[:], c2_d.rearrange("(m p) -> p m", p=P))
        if has_eb:
            ebr = cp.tile([1, NT * N2], BF16, tag="ebr")
            nc.sync.dma_start(ebr[:], ebr_d[:])
            ones512 = cp.tile([1, CHUNK], BF16, tag="ones512")
            nc.vector.memset(ones512[:], 1.0)
        if has_b3:
            b3r = cp.tile([1, OUTW], BF16, tag="b3r")
            nc.sync.dma_start(b3r[:], b3r_d[:])
        ones1 = cp.tile([1, P], BF16, tag="ones1")
        nc.vector.memset(ones1[:], 1.0)
        ident = cp.tile([P, P], F32, tag="ident")
        make_identity(nc, ident[:])
        identb = cp.tile([P, P], BF16, tag="identb")
        make_identity(nc, identb[:])
        # selmat[:, t*P:(t+1)*P] is [17,128] with row 1+t all-ones: a K=17
        # matmul against nmT adds expert t's additive mask row to PSUM.
        selmat = cp.tile([1 + NT, NT * P], BF16, tag="selmat")
        nc.sync.dma_start(selmat[:], selm_d[:])

        # ---- working pools ----
        xbf_p = pool("xbf", 4)        # 4 x [128, QW] bf16 cast staging
        xT_p = pool("xT", 8)
        ek_p = pool("ek", 4)          # dose-row expert weights [1,128]
        td_p = pool("tdp", 2)
        pk_p = pool("pk", 2)
        nm_p = pool("nm", 2)          # [1+NT, CHUNK]: dose + mask rows
        h1_p = pool("h1", 1)
        emb_p = pool("emb", 1)
        lat_p = pool("lat", 1)
        latb_p = pool("latb", 1)
        lo_p = pool("lo", 2)
        d1_p = pool("d1", 1)
        d2_p = pool("d2", 1)
        os_p = pool("os", 2)
        pmain = pool("pmain", 4, "PSUM")
        pout = pool("pout", 2, "PSUM")
        paux = pool("paux", 2, "PSUM")

        for c in range(nchunk):
            r0 = c * CHUNK

            # -- routing: dose + additive mask rows (transposed via PE).
            # nmT row 0 = dose, rows 1..16 = per-expert additive mask
            # (0 on the routed column, -1e6 elsewhere). All writers are
            # DVE copies so PE consumers need a single semaphore wait.
            nmT = nm_p.tile([1 + NT, CHUNK], BF16, tag="nm")
            for bb in range(4):
                tdt = td_p.tile([P, NT], F32, tag="tdp")
                nc.scalar.dma_start(tdt[:], td_d[r0 + bb * P:r0 + (bb + 1) * P, :])
                pk = pk_p.tile([P, 1 + NT], F32, tag="pk")
                nc.vector.reduce_max(pk[:, 0:1], tdt[:], axis=mybir.AxisListType.X)
                # (td < rowmax) * -1e6 -> 0 on the argmax column, -1e6 else
                nc.vector.tensor_scalar(pk[:, 1:1 + NT], tdt[:], pk[:, 0:1],
                                        NEG, ALU.is_lt, ALU.mult)
                pt = paux.tile([P, P], F32, tag="paux", name="pt_route")
                nc.tensor.transpose(pt[0:1 + NT, :], pk[:], ident[:])
                nc.vector.tensor_copy(nmT[:, bb * P:(bb + 1) * P],
                                      pt[0:1 + NT, :])

            # -- input: cast-DMA quarters, DMA-transpose, enc1 matmuls --
            ps1 = [pmain.tile([P, CHUNK], F32, tag="pmain", name=f"ps1_{m}")
                   for m in range(4)]
            for q in range(K1 // QK):
                xbfs = []
                for bb in range(4):
                    xbf = xbf_p.tile([P, QW], BF16, tag="xbf")
                    nc.gpsimd.dma_start(
                        xbf[:], x_d[r0 + bb * P:r0 + (bb + 1) * P,
                                    q * QW:(q + 1) * QW])
                    xbfs.append(xbf)
                for kk in range(QK):
                    k = q * QK + kk
                    xT = xT_p.tile([P, CHUNK], BF16, tag="xT")
                    for bb in range(4):
                        ptx = paux.tile([P, P], BF16, tag="paux",
                                        name="ptx")
                        nc.tensor.transpose(ptx[:],
                                            xbfs[bb][:, kk * P:(kk + 1) * P],
                                            identb[:])
                        nc.vector.tensor_copy(xT[:, bb * P:(bb + 1) * P],
                                              ptx[:])
                    for m in range(4):
                        nc.tensor.matmul(ps1[m][:],
                                         w1r[:, k, m * P:(m + 1) * P], xT[:],
                                         start=(k == 0), stop=(k == K1 - 1))
            h1T = h1_p.tile([P, 4, CHUNK], BF16, tag="h1")
            for m in range(4):
                nc.scalar.activation(h1T[:, m, :], ps1[m][:], AF.Relu,
                                     bias=b1r[:, m:m + 1])

            # -- enc2 --
            embT = emb_p.tile([P, 2, CHUNK], BF16, tag="emb")
            for m in range(2):
                p2 = pmain.tile([P, CHUNK], F32, tag="pmain")
                for k in range(4):
                    nc.tensor.matmul(p2[:], w2r[:, k, m * P:(m + 1) * P],
                                     h1T[:, k, :], start=(k == 0), stop=(k == 3))
                nc.scalar.activation(embT[:, m, :], p2[:], AF.Relu,
                                     bias=b2r[:, m:m + 1])

            # -- experts: compute-all + additive-mask max-select --
            latsel = lat_p.tile([P, 2, CHUNK], F32, tag="lat")
            for t in range(NT):
                for m in range(2):
                    ek = ek_p.tile([1, P], BF16, tag="ek")
                    nc.scalar.dma_start(ek[:], ew_d[t, N2, m * P:(m + 1) * P])
                    pe = pmain.tile([P, CHUNK], F32, tag="pmain")
                    nc.tensor.matmul(pe[:], ewr[:, t, 0, m * P:(m + 1) * P],
                                     embT[:, 0, :], start=True, stop=False)
                    nc.tensor.matmul(pe[:], ewr[:, t, 1, m * P:(m + 1) * P],
                                     embT[:, 1, :], start=False, stop=False)
                    nc.tensor.matmul(pe[:], ek[:], nmT[0:1, :],
                                     start=False, stop=False)
                    if has_eb:
                        nc.tensor.matmul(
                            pe[:], ebr[0:1, t * N2 + m * P:t * N2 + (m + 1) * P],
                            ones512[0:1, :], start=False, stop=False)
                    nc.tensor.matmul(pe[:], selmat[:, t * P:(t + 1) * P],
                                     nmT[:], start=False, stop=True)
                    if t == 0:
                        nc.vector.tensor_copy(latsel[:, m, :], pe[:])
                    else:
                        nc.vector.tensor_tensor(latsel[:, m, :], latsel[:, m, :],
                                                pe[:], op=ALU.max)

            # -- latent: bf16 relu for decoder; fp32 relu via transpose out --
            latbf = latb_p.tile([P, 2, CHUNK], BF16, tag="latb")
            for m in range(2):
                nc.scalar.activation(latbf[:, m, :], latsel[:, m, :], AF.Relu)
            for bb in range(4):
                lo = lo_p.tile([P, N2], F32, tag="lo")
                for m in range(2):
                    pt = paux.tile([P, P], F32, tag="paux", name="pt_lat")
                    nc.tensor.transpose(pt[:], latsel[:, m, bb * P:(bb + 1) * P],
                                        ident[:])
                    nc.scalar.activation(lo[:, m * P:(m + 1) * P], pt[:], AF.Relu)
                nc.scalar.dma_start(lat_d[r0 + bb * P:r0 + (bb + 1) * P, :], lo[:])

            # -- dec1 / dec2 --
            d1T = d1_p.tile([P, 4, CHUNK], BF16, tag="d1")
            for m in range(4):
                p = pmain.tile([P, CHUNK], F32, tag="pmain")
                for k in range(2):
                    nc.tensor.matmul(p[:], wd1r[:, k, m * P:(m + 1) * P],
                                     latbf[:, k, :], start=(k == 0), stop=(k == 1))
                nc.scalar.activation(d1T[:, m, :], p[:], AF.Relu,
                                     bias=c1r[:, m:m + 1])
            d2T = d2_p.tile([P, 4, CHUNK], BF16, tag="d2")
            for m in range(4):
                p = pmain.tile([P, CHUNK], F32, tag="pmain")
                for k in range(4):
                    nc.tensor.matmul(p[:], wd2r[:, k, m * P:(m + 1) * P],
                                     d1T[:, k, :], start=(k == 0), stop=(k == 3))
                nc.scalar.activation(d2T[:, m, :], p[:], AF.Relu,
                                     bias=c2r[:, m:m + 1])

            # -- dec3: batch-major output + means/vars store --
            for bb in range(4):
                for j in range(OUTW // NW):
                    po = pout.tile([P, NW], F32, tag="pout")
                    for k in range(4):
                        nc.tensor.matmul(po[:], d2T[:, k, bb * P:(bb + 1) * P],
                                         wd3r[:, k, j * NW:(j + 1) * NW],
                                         start=(k == 0),
                                         stop=(k == 3 and not has_b3))
                    if has_b3:
                        nc.tensor.matmul(po[:], ones1[0:1, :],
                                         b3r[0:1, j * NW:(j + 1) * NW],
                                         start=False, stop=True)
                    os = os_p.tile([P, NW], F32, tag="os")
                    if j < IN // NW:
                        nc.scalar.copy(os[:], po[:])
                    else:
                        # softplus = ln(exp(x) + 1); Ln's bias is pre-func
                        nc.scalar.activation(os[:], po[:], AF.Exp)
                        nc.scalar.activation(os[:], os[:], AF.Ln, bias=1.0)
                        nc.vector.tensor_scalar_add(os[:], os[:], 0.001)
                    nc.scalar.dma_start(
                        rec_d[r0 + bb * P:r0 + (bb + 1) * P, j * NW:(j + 1) * NW],
                        os[:])
    nc.compile()
    return nc


_BF = ml_dtypes.bfloat16

SELM = np.zeros((1 + NT, NT * P), dtype=_BF)
for _t in range(NT):
    SELM[1 + _t, _t * P:(_t + 1) * P] = 1.0


def kernel(input, treatment_and_dosages, enc_w1, enc_b1, enc_w2, enc_b2,
           expert_w, expert_b, dec_w1, dec_b1, dec_w2, dec_b2, dec_w3, dec_b3):
    global LAST_EXEC_NS, LAST_RESULTS
    x = np.asarray(input, dtype=np.float32)
    td = np.ascontiguousarray(np.asarray(treatment_and_dosages, dtype=np.float32))
    assert x.shape == (BATCH, IN) and td.shape == (BATCH, NT)
    xp = np.zeros((BATCH, INP), dtype=np.float32)
    xp[:, :IN] = x
    w1 = np.zeros((INP, N0), dtype=_BF)
    w1[:IN, :] = np.asarray(enc_w1, dtype=np.float32).astype(_BF)
    w2 = np.asarray(enc_w2, dtype=np.float32).astype(_BF)
    ew = np.ascontiguousarray(np.asarray(expert_w, dtype=np.float32).astype(_BF))
    wd1 = np.asarray(dec_w1, dtype=np.float32).astype(_BF)
    wd2 = np.asarray(dec_w2, dtype=np.float32).astype(_BF)
    wd3 = np.ascontiguousarray(np.asarray(dec_w3, dtype=np.float32).astype(_BF))
    b1 = np.ascontiguousarray(np.asarray(enc_b1, dtype=np.float32))
    b2 = np.ascontiguousarray(np.asarray(enc_b2, dtype=np.float32))
    eb = np.asarray(expert_b, dtype=np.float32)
    c1 = np.ascontiguousarray(np.asarray(dec_b1, dtype=np.float32))
    c2 = np.ascontiguousarray(np.asarray(dec_b2, dtype=np.float32))
    c3 = np.asarray(dec_b3, dtype=np.float32)
    has_eb = bool(np.any(eb))
    has_b3 = bool(np.any(c3))

    nc = build_program(has_eb, has_b3, ROWS // CHUNK)

    in_maps = []
    for i in range(NCORES):
        r = slice(i * ROWS, (i + 1) * ROWS)
        m = {"x": xp[r], "td": td[r], "w1": w1, "b1": b1, "w2": w2, "b2": b2,
             "ew": ew, "wd1": wd1, "c1": c1, "wd2": wd2, "c2": c2, "wd3": wd3,
             "selm": SELM}
        if has_eb:
            m["ebr"] = np.ascontiguousarray(eb.astype(_BF).reshape(1, NT * N2))
        if has_b3:
            m["b3r"] = np.ascontiguousarray(c3.astype(_BF).reshape(1, OUTW))
        in_maps.append(m)

    res = run_bass_kernel_spmd(nc, in_maps, list(range(NCORES)))
    LAST_RESULTS = res
    LAST_EXEC_NS = res.exec_time_ns
    rec = np.concatenate([res.results[i]["rec"] for i in range(NCORES)], axis=0)
    lat = np.concatenate([res.results[i]["lat"] for i in range(NCORES)], axis=0)
    return rec, lat
